# revision 1
# baseline (speedup 1.0000x reference)
"""Trainium2 Bass kernel for nn_DeepMapping2D (histogram_binning).

Reference semantics: per cloud, quantize points to integer mm bins
(q = round_half_even(1000*p)), histogram into a 1024x1024 grid (shifted by
per-cloud coordinate minima), threshold counts (count/N > 2e-4 <=> count>=53),
sort the 0/1 occupancy descending, truncate to TOPK.  The sorted vector is K
ones then zeros, K = #bins with count >= 53.  Shifting by the minima is a
bijection on occupied bins, so K is shift-invariant and the device can work
on unshifted bin ids s = qx*1024 + qz (fine id, < 2^20).

Device algorithm (exact, two launches, all heavy work on device):

Phase 1 (screen): per cloud, the exact 2^14-bin coarse histogram H14 over
c14 = s>>6, computed as a PSUM-matmul scatter: per column of 128 points,
build 128-wide one-hots of hi7=c14>>7 and lo7=c14&127 by comparing a
constant iota row against the point's value (DVE tensor_scalar is_equal with
a per-partition scalar), then accumulate onehot_hi^T @ onehot_lo into PSUM
(bf16 0/1 inputs are exact; fp32 accumulation).  H14 goes back to the host.

Host: candidate cells = {c14 : H14[c14] >= 53} (every fine bin with count
>= 53 lives in one, since H14 upper-bounds its 64 fine bins).  ~1.4k/cloud
for the rbg-generated inputs.  Sorted, padded with -1 to NCHUNK*128 int16.

Phase 2 (refine): per cloud, exact fine counts for every candidate cell:
per column, one membership one-hot against the candidate row (int16
candidates streamed at DVE 4x, compared against the point's c14 as the
per-partition scalar) and one 64-wide one-hot of low6 = s&63; NCHUNK
matmuls accumulate membership^T @ onehot_low6 into PSUM -> exact
[candidate, low6] fine counts.  Threshold >= 53, count via ones^T @ mask
matmul, giving K per cloud.  The host formats the final rows (K ones then
zeros) from the device-computed K values.

Host guards keep the kernel exact for arbitrary inputs: clouds whose
candidate count exceeds capacity (or phase-2 K disagreeing with impossible
states) fall back to an exact numpy recomputation of that cloud.

Sharding: data-parallel over batch: 64 clouds -> 8 cores x 8 clouds.
"""

import numpy as np

B = 64
N = 262144
TOPK = 5120
NCORES = 8
CLOUDS_PER_CORE = B // NCORES
P = 128
GZ = 1024
NCHUNK = 12  # candidate capacity = NCHUNK*128 cells per cloud
CAND_CAP = NCHUNK * P
THRESH_COUNT = 53.0
C23 = 12582912.0  # 1.5 * 2^23

_cache = {}


def _chain(nc, tc, pools, pcd, c, F, mybir, need_low6, col0=0, ftot=None):
    """Elementwise chain for columns [col0, col0+F) of cloud c."""
    import concourse.bass as bass

    f32 = mybir.dt.float32
    op = mybir.AluOpType
    workp, chainp = pools
    if ftot is None:
        ftot = F

    tin = workp.tile([P, 2 * F], f32, tag="tin")
    src = pcd[c].rearrange("(p f) t -> p (f t)", p=P)
    nc.gpsimd.dma_start(out=tin[:], in_=src[:, 2 * col0 : 2 * (col0 + F)])
    # q = round_half_even(1000*p)  (the +/- 1.5*2^23 trick == jnp.round)
    tt = chainp.tile([P, 2 * F], f32, tag="tt")
    nc.vector.tensor_scalar(
        out=tt[:], in0=tin[:], scalar1=1000.0, scalar2=C23, op0=op.mult, op1=op.add
    )
    tq = chainp.tile([P, 2 * F], f32, tag="tq")
    nc.vector.tensor_scalar(
        out=tq[:], in0=tt[:], scalar1=C23, scalar2=None, op0=op.subtract
    )
    # s = qx*1024 + qz (exact, < 2^24)
    tq3 = tq[:].rearrange("p (f t) -> p t f", t=2)
    ts_ = chainp.tile([P, F], f32, tag="ts")
    nc.vector.scalar_tensor_tensor(
        out=ts_[:], in0=tq3[:, 0], scalar=1024.0, in1=tq3[:, 1],
        op0=op.mult, op1=op.add,
    )
    # c14 = floor(s/64): s*2^-6 is exact, offset by -63/128 (exact), then the
    # fused (+C23, -C23) forces a round-to-nearest at integer granularity.
    tu = chainp.tile([P, F], f32, tag="tu")
    nc.vector.tensor_scalar(
        out=tu[:], in0=ts_[:], scalar1=0.015625, scalar2=0.4921875,
        op0=op.mult, op1=op.subtract,
    )
    tc14 = workp.tile([P, F], f32, tag="tc14")
    nc.vector.tensor_scalar(
        out=tc14[:], in0=tu[:], scalar1=C23, scalar2=C23, op0=op.add, op1=op.subtract
    )
    tlow6 = None
    if need_low6:
        # low6 = s - 64*c14
        tlow6 = workp.tile([P, F], f32, tag="tlow6")
        nc.vector.scalar_tensor_tensor(
            out=tlow6[:], in0=tc14[:], scalar=-64.0, in1=ts_[:],
            op0=op.mult, op1=op.add,
        )
    return tc14, tlow6, ts_


def build_phase1(n_clouds=CLOUDS_PER_CORE, n_points=N, unroll=32):
    """Per-cloud exact 2^14-bin coarse histogram -> DRAM."""
    import concourse.bass as bass
    import concourse.mybir as mybir
    from concourse.tile import TileContext

    f32, bf16, i32 = mybir.dt.float32, mybir.dt.bfloat16, mybir.dt.int32
    op = mybir.AluOpType
    F = n_points // P

    from concourse import bacc

    nc = bacc.Bacc("TRN2", target_bir_lowering=False, debug=False)
    pcd = nc.declare_dram_parameter("pcd", [n_clouds, n_points, 2], f32, isOutput=False)
    h14 = nc.declare_dram_parameter("h14", [n_clouds, P, P], f32, isOutput=True)

    with TileContext(nc) as tc:
        with (
            tc.tile_pool(name="const", bufs=1) as constp,
            tc.tile_pool(name="work", bufs=2) as workp,
            tc.tile_pool(name="chain", bufs=1) as chainp,
            tc.tile_pool(name="hilo", bufs=1) as hilop,
            tc.tile_pool(name="oh", bufs=8) as ohp,
            tc.tile_pool(name="hout", bufs=2) as houtp,
            tc.tile_pool(name="psum", bufs=1, space="PSUM") as psump,
        ):
            iota_i = constp.tile([P, P], i32)
            nc.gpsimd.iota(iota_i[:], pattern=[[1, P]], base=0, channel_multiplier=0)
            iota_bf = constp.tile([P, P], bf16)
            nc.vector.tensor_copy(out=iota_bf[:], in_=iota_i[:])

            this, tlos, hists = [], [], []
            FC = min(512, F)  # chain chunk width (columns)
            for c in range(n_clouds):
                thi = hilop.tile([P, F], f32, tag=f"thi{c}")
                tlo = hilop.tile([P, F], f32, tag=f"tlo{c}")
                for col0 in range(0, F, FC):
                    tc14, _, _ = _chain(
                        nc, tc, (workp, chainp), pcd, c, FC, mybir,
                        need_low6=False, col0=col0, ftot=F,
                    )
                    # hi7 = floor(c14/128); lo7 = c14 - 128*hi7
                    thif = chainp.tile([P, FC], f32, tag="thif")
                    nc.vector.tensor_scalar(
                        out=thif[:], in0=tc14[:], scalar1=0.0078125,
                        scalar2=0.49609375, op0=op.mult, op1=op.subtract,
                    )
                    sl = slice(col0, col0 + FC)
                    nc.vector.tensor_scalar(
                        out=thi[:, sl], in0=thif[:], scalar1=C23, scalar2=C23,
                        op0=op.add, op1=op.subtract,
                    )
                    nc.vector.scalar_tensor_tensor(
                        out=tlo[:, sl], in0=thi[:, sl], scalar=-128.0,
                        in1=tc14[:], op0=op.mult, op1=op.add,
                    )
                this.append(thi)
                tlos.append(tlo)
                hist = psump.tile([P, P], f32, tag=f"hist{c}")
                nc.vector.memset(hist[:], 0.0)
                hists.append(hist)

            def body(iv):
                for c in range(n_clouds):
                    ohh = ohp.tile([P, P], bf16, tag="ohh")
                    ohl = ohp.tile([P, P], bf16, tag="ohl")
                    nc.vector.tensor_scalar(
                        out=ohh[:], in0=iota_bf[:],
                        scalar1=this[c][:, bass.ds(iv, 1)], scalar2=None,
                        op0=op.is_equal,
                    )
                    nc.vector.tensor_scalar(
                        out=ohl[:], in0=iota_bf[:],
                        scalar1=tlos[c][:, bass.ds(iv, 1)], scalar2=None,
                        op0=op.is_equal,
                    )
                    nc.tensor.matmul(
                        out=hists[c][:], lhsT=ohh[:], rhs=ohl[:],
                        start=False, stop=True, skip_group_check=True,
                    )

            tc.For_i_unrolled(0, F, 1, body, max_unroll=unroll)

            for c in range(n_clouds):
                hsb = houtp.tile([P, P], f32, tag="hsb")
                nc.vector.tensor_copy(out=hsb[:], in_=hists[c][:])
                nc.gpsimd.dma_start(out=h14[c], in_=hsb[:])
    nc.compile()
    return nc


def build_phase2(n_clouds=CLOUDS_PER_CORE, n_points=N, nchunk=NCHUNK, unroll=16):
    """Exact [candidate,64] fine counts + K per cloud + output rows."""
    import concourse.bass as bass
    import concourse.mybir as mybir
    from concourse.tile import TileContext

    f32, bf16 = mybir.dt.float32, mybir.dt.bfloat16
    i16, i32 = mybir.dt.int16, mybir.dt.int32
    op = mybir.AluOpType
    F = n_points // P
    cap = nchunk * P

    from concourse import bacc

    nc = bacc.Bacc("TRN2", target_bir_lowering=False, debug=False)
    pcd = nc.declare_dram_parameter("pcd", [n_clouds, n_points, 2], f32, isOutput=False)
    cands = nc.declare_dram_parameter("cands", [n_clouds, cap], i16, isOutput=False)
    kvals = nc.declare_dram_parameter("kvals", [1, n_clouds], f32, isOutput=True)
    outr = nc.declare_dram_parameter("outr", [n_clouds, TOPK], f32, isOutput=True)
    kscr = nc.dram_tensor("kscr", [n_clouds], f32)

    with TileContext(nc) as tc:
        with (
            tc.tile_pool(name="const", bufs=1) as constp,
            tc.tile_pool(name="work", bufs=2) as workp,
            tc.tile_pool(name="chain", bufs=1) as chainp,
            tc.tile_pool(name="oh", bufs=8) as ohp,
            tc.tile_pool(name="mk", bufs=4) as mkp,
            tc.tile_pool(name="psum", bufs=1, space="PSUM") as psump,
            tc.tile_pool(name="kps", bufs=1, space="PSUM") as kpsp,
        ):
            iota64_i = constp.tile([P, 64], i32)
            nc.gpsimd.iota(iota64_i[:], pattern=[[1, 64]], base=0, channel_multiplier=0)
            iota64_bf = constp.tile([P, 64], bf16)
            nc.vector.tensor_copy(out=iota64_bf[:], in_=iota64_i[:])
            ones_bf = constp.tile([P, 1], bf16)
            nc.vector.memset(ones_bf[:], 1.0)
            iota5k_i = constp.tile([P, TOPK], i32)
            nc.gpsimd.iota(
                iota5k_i[:], pattern=[[1, TOPK]], base=0, channel_multiplier=0
            )
            iota5k_f = constp.tile([P, TOPK], f32)
            nc.vector.tensor_copy(out=iota5k_f[:], in_=iota5k_i[:])
            kv_sb = constp.tile([1, n_clouds], f32)

            for c in range(n_clouds):
                tc14, tlow6, _ = _chain(
                    nc, tc, (workp, chainp), pcd, c, F, mybir, need_low6=True
                )
                # candidate row broadcast to all partitions
                candbc = workp.tile([P, cap], i16, tag="candbc")
                cand_src = bass.AP(
                    tensor=cands.tensor if hasattr(cands, "tensor") else cands,
                    offset=c * cap,
                    ap=[[0, P], [1, cap]],
                )
                nc.gpsimd.dma_start(out=candbc[:], in_=cand_src)

                hist = psump.tile([P, cap], f32, tag="hist")
                nc.vector.memset(hist[:], 0.0)

                def body(iv):
                    memb = ohp.tile([P, cap], bf16, tag="memb")
                    loh = ohp.tile([P, 64], bf16, tag="loh")
                    nc.vector.tensor_scalar(
                        out=memb[:], in0=candbc[:],
                        scalar1=tc14[:, bass.ds(iv, 1)], scalar2=None,
                        op0=op.is_equal,
                    )
                    nc.vector.tensor_scalar(
                        out=loh[:], in0=iota64_bf[:],
                        scalar1=tlow6[:, bass.ds(iv, 1)], scalar2=None,
                        op0=op.is_equal,
                    )
                    # transposed accumulation: hist[w, cand] += loh^T @ memb,
                    # 512-wide moving slices so the 64-wide stationary loh is
                    # shared and PE streams at full width
                    for g in range(cap // 512):
                        nc.tensor.matmul(
                            out=hist[:64, g * 512 : (g + 1) * 512],
                            lhsT=loh[:],
                            rhs=memb[:, g * 512 : (g + 1) * 512],
                            start=False, stop=True, skip_group_check=True,
                        )

                tc.For_i_unrolled(0, F, 1, body, max_unroll=unroll)

                # K = sum over candidates/low6 of [count >= 53]
                kps = kpsp.tile([1, cap], f32, tag="kps")
                for g in range(cap // 512):
                    mask = mkp.tile([P, 512], bf16, tag="mask")
                    nc.vector.tensor_scalar(
                        out=mask[:64, :], in0=hist[:64, g * 512 : (g + 1) * 512],
                        scalar1=52.5, scalar2=None, op0=op.is_ge,
                    )
                    nc.tensor.matmul(
                        out=kps[:1, g * 512 : (g + 1) * 512],
                        lhsT=ones_bf[:64, :], rhs=mask[:64, :],
                        start=True, stop=True,
                    )
                nc.vector.tensor_reduce(
                    out=kv_sb[:1, c : c + 1], in_=kps[:],
                    axis=mybir.AxisListType.X, op=op.add,
                )

            nc.gpsimd.dma_start(out=kvals[:, :], in_=kv_sb[:])
            # output rows = (iota5120 < K) per cloud.  K values live on
            # partition 0; roundtrip through DRAM to spread one per partition.
            if n_clouds == 1:
                kcol = kv_sb
            else:
                nc.gpsimd.dma_start(out=kscr[:], in_=kv_sb[0, :])
                kcol = constp.tile([n_clouds, 1], f32)
                nc.gpsimd.dma_start(
                    out=kcol[:], in_=kscr[:].rearrange("(b o) -> b o", o=1)
                )
            orow = constp.tile([n_clouds, TOPK], f32)
            nc.vector.tensor_scalar(
                out=orow[:], in0=iota5k_f[:n_clouds, :],
                scalar1=kcol[:n_clouds, 0:1], scalar2=None, op0=op.is_lt,
            )
            nc.gpsimd.dma_start(out=outr[:, :], in_=orow[:])
    nc.compile()
    return nc


def _host_exact(points):
    """Exact numpy replica of the reference for one cloud. [N,2] f32 -> [TOPK]."""
    q = np.round(np.float32(1000.0) * points.astype(np.float32))
    xi = (q[:, 0] - q[:, 0].min()).astype(np.int64)
    zi = (q[:, 1] - q[:, 1].min()).astype(np.int64)
    idx = xi * GZ + zi
    counts = np.bincount(idx, minlength=1024 * GZ).astype(np.float32)
    occ = counts / np.float32(points.shape[0]) > np.float32(0.0002)
    k = min(int(occ.sum()), TOPK)
    out = np.zeros((TOPK,), np.float32)
    out[:k] = 1.0
    return out


def _modules():
    if "m" not in _cache:
        _cache["m"] = (build_phase1(), build_phase2())
    return _cache["m"]


def kernel(pcd):
    from concourse.bass_utils import run_bass_kernel_spmd

    pcd = np.ascontiguousarray(np.asarray(pcd), dtype=np.float32)
    assert pcd.shape == (B, N, 2), pcd.shape
    nc1, nc2 = _modules()
    shards = pcd.reshape(NCORES, CLOUDS_PER_CORE, N, 2)
    core_ids = list(range(NCORES))

    res1 = run_bass_kernel_spmd(nc1, [{"pcd": shards[i]} for i in range(NCORES)], core_ids)

    cand_arrays = []
    overflow = {}  # (core, cloud) -> True
    for i in range(NCORES):
        h14 = np.asarray(res1.results[i]["h14"], np.float32).reshape(
            CLOUDS_PER_CORE, P * P
        )
        carr = np.full((CLOUDS_PER_CORE, CAND_CAP), -1, np.int16)
        for c in range(CLOUDS_PER_CORE):
            cand = np.nonzero(h14[c] >= THRESH_COUNT)[0]
            if len(cand) > CAND_CAP:
                overflow[(i, c)] = True
            else:
                carr[c, : len(cand)] = cand.astype(np.int16)
        cand_arrays.append(carr)

    res2 = run_bass_kernel_spmd(
        nc2,
        [{"pcd": shards[i], "cands": cand_arrays[i]} for i in range(NCORES)],
        core_ids,
    )
    # Assemble: K ones then zeros per cloud, from the device-computed K
    # (trivial formatting; the histogramming/thresholding all ran on device).
    out = np.zeros((B, TOPK, 1), np.float32)
    iota = np.arange(TOPK)
    for i in range(NCORES):
        kv = np.asarray(res2.results[i]["kvals"], np.float32).reshape(-1)
        for c in range(CLOUDS_PER_CORE):
            b = i * CLOUDS_PER_CORE + c
            if (i, c) in overflow:
                out[b, :, 0] = _host_exact(shards[i, c])
            else:
                out[b, :, 0] = (iota < kv[c]).astype(np.float32)
    return out



# revision 6
# speedup vs baseline: 3.1156x; 3.1156x over previous
"""Trainium2 Bass kernel for nn_DeepMapping2D (histogram_binning).

Reference semantics: per cloud, quantize points to integer mm bins
(q = round_half_even(1000*p)), histogram into a 1024x1024 grid (shifted by
per-cloud coordinate minima), threshold counts (count/N > 2e-4 <=> count>=53),
sort the 0/1 occupancy descending, truncate to TOPK.  The sorted vector is K
ones then zeros, K = #bins with count >= 53.  Shifting by the minima is a
bijection on occupied bins, so K is shift-invariant and the device can work
on unshifted bin ids s = qx*1024 + qz (fine id, < 2^20).

Device algorithm (exact, two launches, all heavy counting on device):

Phase 1 (screen): per cloud, the exact 2^14-bin coarse histogram H14 over
c14 = s>>6, computed as a PSUM-matmul scatter: per column of 128 points,
build 128-wide one-hots of hi7=c14>>7 and lo7=c14&127 by comparing a
constant iota row against the point's value (DVE tensor_scalar is_equal with
a per-partition scalar), then accumulate onehot_hi^T @ onehot_lo into PSUM
(bf16 0/1 inputs are exact; fp32 accumulation).  H14, clamped to u8, goes
back to the host (1 MB total).

Host: candidate cells = {c14 : H14[c14] >= 53} (every fine bin with count
>= 53 lives in one, since H14 upper-bounds its 64 fine bins).  ~1.4k/cloud
for the rbg-generated inputs.  Padded with -1 to NCHUNK*128 int16.

Phase 2 (refine): per cloud, exact fine counts for every candidate cell:
per column, one membership one-hot against the candidate row (int16
candidates streamed at DVE 4x, compared against the point's c14 as the
per-partition scalar) and one 64-wide one-hot of low6 = s&63; NCHUNK
matmuls accumulate membership^T @ onehot_low6 into PSUM -> exact
[candidate, low6] fine counts.  Threshold >= 53, count via ones^T @ mask
matmul, giving K per cloud.  The host formats the final rows (K ones then
zeros) from the device-computed K values.

Transport optimization (the axon tunnel runs at ~35 MB/s, so bytes moved
dominate wall time): the host quantizes once into two integer planes,
c14 = s>>6 (uint16) and lo6 = s&63 (uint8) - 3 B/point = 50 MB instead of
the 8 B/point raw floats - and uploads them a single time.  Both phases
run through a jit(shard_map(bass_exec)) callable (the same primitive
bass_utils.run_bass_kernel_spmd lowers to under axon) against the SAME
device-resident plane arrays, so phase 2 re-reads them from device DRAM
instead of re-shipping 128 MB.  Quantization is pipelined per core-shard
with the uploads.

Host guards keep the kernel exact for arbitrary inputs: clouds with
coordinates outside [0, 1023] mm (or counts exceeding the candidate
capacity) fall back to an exact numpy recomputation of that cloud.

Sharding: data-parallel over batch: 64 clouds -> 8 cores x 8 clouds.
"""

import os
import numpy as np

B = 64
N = 262144
TOPK = 5120
NCORES = 8
CLOUDS_PER_CORE = B // NCORES
P = 128
F = N // P
GZ = 1024
NCHUNK = 12  # candidate capacity = NCHUNK*128 cells per cloud
CAND_CAP = NCHUNK * P
THRESH_COUNT = 53.0
C23 = 12582912.0  # 1.5 * 2^23

_cache = {}
_DEBUG = os.environ.get("KERNEL_DEBUG", "0") == "1"


def _dbg(msg, t0=None):
    if _DEBUG:
        import time

        if t0 is not None:
            print(f"[kernel] {msg}: {time.time()-t0:.3f}s", flush=True)
        else:
            print(f"[kernel] {msg}", flush=True)


def build_phase1(n_clouds=CLOUDS_PER_CORE, n_points=N, unroll=32):
    """Per-cloud exact 2^14-bin coarse histogram -> DRAM (u8, clamped)."""
    import concourse.bass as bass
    import concourse.mybir as mybir
    from concourse.tile import TileContext
    from concourse import bacc

    f32, bf16 = mybir.dt.float32, mybir.dt.bfloat16
    i32, u16, u8 = mybir.dt.int32, mybir.dt.uint16, mybir.dt.uint8
    op = mybir.AluOpType
    Fl = n_points // P

    nc = bacc.Bacc("TRN2", target_bir_lowering=False, debug=False)
    c14p = nc.declare_dram_parameter("c14p", [n_clouds, n_points], u16, isOutput=False)
    h14 = nc.declare_dram_parameter("h14", [n_clouds, P, P], u8, isOutput=True)

    with TileContext(nc) as tc:
        with (
            tc.tile_pool(name="const", bufs=1) as constp,
            tc.tile_pool(name="raw", bufs=2) as rawp,
            tc.tile_pool(name="chain", bufs=2) as chainp,
            tc.tile_pool(name="hilo", bufs=1) as hilop,
            tc.tile_pool(name="oh", bufs=8) as ohp,
            tc.tile_pool(name="hout", bufs=2) as houtp,
            tc.tile_pool(name="psum", bufs=1, space="PSUM") as psump,
        ):
            iota_i = constp.tile([P, P], i32)
            nc.gpsimd.iota(iota_i[:], pattern=[[1, P]], base=0, channel_multiplier=0)
            iota_bf = constp.tile([P, P], bf16)
            nc.vector.tensor_copy(out=iota_bf[:], in_=iota_i[:])

            this, tlos, hists = [], [], []
            for c in range(n_clouds):
                rc = rawp.tile([P, Fl], u16, tag="rc")
                nc.gpsimd.dma_start(out=rc[:], in_=c14p[c].rearrange("(p f) -> p f", p=P))
                tc14 = chainp.tile([P, Fl], f32, tag="tc14")
                nc.vector.tensor_copy(out=tc14[:], in_=rc[:])
                # hi7 = floor(c14/128); lo7 = c14 - 128*hi7
                thif = chainp.tile([P, Fl], f32, tag="thif")
                nc.vector.tensor_scalar(
                    out=thif[:], in0=tc14[:], scalar1=0.0078125,
                    scalar2=0.49609375, op0=op.mult, op1=op.subtract,
                )
                thi = hilop.tile([P, Fl], f32, tag=f"thi{c}")
                nc.vector.tensor_scalar(
                    out=thi[:], in0=thif[:], scalar1=C23, scalar2=C23,
                    op0=op.add, op1=op.subtract,
                )
                tlo = hilop.tile([P, Fl], f32, tag=f"tlo{c}")
                nc.vector.scalar_tensor_tensor(
                    out=tlo[:], in0=thi[:], scalar=-128.0, in1=tc14[:],
                    op0=op.mult, op1=op.add,
                )
                this.append(thi)
                tlos.append(tlo)
                hist = psump.tile([P, P], f32, tag=f"hist{c}")
                nc.vector.memset(hist[:], 0.0)
                hists.append(hist)

            def body(iv):
                for c in range(n_clouds):
                    ohh = ohp.tile([P, P], bf16, tag="ohh")
                    ohl = ohp.tile([P, P], bf16, tag="ohl")
                    nc.vector.tensor_scalar(
                        out=ohh[:], in0=iota_bf[:],
                        scalar1=this[c][:, bass.ds(iv, 1)], scalar2=None,
                        op0=op.is_equal,
                    )
                    nc.vector.tensor_scalar(
                        out=ohl[:], in0=iota_bf[:],
                        scalar1=tlos[c][:, bass.ds(iv, 1)], scalar2=None,
                        op0=op.is_equal,
                    )
                    nc.tensor.matmul(
                        out=hists[c][:], lhsT=ohh[:], rhs=ohl[:],
                        start=False, stop=True, skip_group_check=True,
                    )

            tc.For_i_unrolled(0, Fl, 1, body, max_unroll=unroll)

            for c in range(n_clouds):
                hcl = houtp.tile([P, P], f32, tag="hcl")
                nc.vector.tensor_scalar(
                    out=hcl[:], in0=hists[c][:], scalar1=255.0, scalar2=None,
                    op0=op.min,
                )
                hu8 = houtp.tile([P, P], u8, tag="hu8")
                nc.vector.tensor_copy(out=hu8[:], in_=hcl[:])
                nc.gpsimd.dma_start(out=h14[c], in_=hu8[:])
    nc.compile()
    return nc


def build_phase2(n_clouds=CLOUDS_PER_CORE, n_points=N, nchunk=NCHUNK, unroll=16):
    """Exact [candidate,64] fine counts -> K per cloud."""
    import concourse.bass as bass
    import concourse.mybir as mybir
    from concourse.tile import TileContext
    from concourse import bacc

    f32, bf16 = mybir.dt.float32, mybir.dt.bfloat16
    i16, i32 = mybir.dt.int16, mybir.dt.int32
    u16, u8 = mybir.dt.uint16, mybir.dt.uint8
    op = mybir.AluOpType
    Fl = n_points // P
    cap = nchunk * P

    nc = bacc.Bacc("TRN2", target_bir_lowering=False, debug=False)
    c14p = nc.declare_dram_parameter("c14p", [n_clouds, n_points], u16, isOutput=False)
    lo6p = nc.declare_dram_parameter("lo6p", [n_clouds, n_points], u8, isOutput=False)
    cands = nc.declare_dram_parameter("cands", [n_clouds, cap], i16, isOutput=False)
    kvals = nc.declare_dram_parameter("kvals", [1, n_clouds], f32, isOutput=True)

    with TileContext(nc) as tc:
        with (
            tc.tile_pool(name="const", bufs=1) as constp,
            tc.tile_pool(name="raw", bufs=2) as rawp,
            tc.tile_pool(name="cloud", bufs=2) as cloudp,
            tc.tile_pool(name="oh", bufs=8) as ohp,
            tc.tile_pool(name="mk", bufs=4) as mkp,
            tc.tile_pool(name="psum", bufs=1, space="PSUM") as psump,
            tc.tile_pool(name="kps", bufs=1, space="PSUM") as kpsp,
        ):
            iota64_i = constp.tile([P, 64], i32)
            nc.gpsimd.iota(iota64_i[:], pattern=[[1, 64]], base=0, channel_multiplier=0)
            iota64_bf = constp.tile([P, 64], bf16)
            nc.vector.tensor_copy(out=iota64_bf[:], in_=iota64_i[:])
            ones_bf = constp.tile([P, 1], bf16)
            nc.vector.memset(ones_bf[:], 1.0)
            kv_sb = constp.tile([1, n_clouds], f32)

            for c in range(n_clouds):
                rc = rawp.tile([P, Fl], u16, tag="rc")
                nc.gpsimd.dma_start(out=rc[:], in_=c14p[c].rearrange("(p f) -> p f", p=P))
                tc14 = cloudp.tile([P, Fl], f32, tag="tc14")
                nc.vector.tensor_copy(out=tc14[:], in_=rc[:])
                rl = rawp.tile([P, Fl], u8, tag="rl")
                nc.gpsimd.dma_start(out=rl[:], in_=lo6p[c].rearrange("(p f) -> p f", p=P))
                tlow6 = cloudp.tile([P, Fl], f32, tag="tlow6")
                nc.vector.tensor_copy(out=tlow6[:], in_=rl[:])

                # candidate row broadcast to all partitions
                candbc = cloudp.tile([P, cap], i16, tag="candbc")
                cand_src = bass.AP(
                    tensor=cands.tensor if hasattr(cands, "tensor") else cands,
                    offset=c * cap,
                    ap=[[0, P], [1, cap]],
                )
                nc.gpsimd.dma_start(out=candbc[:], in_=cand_src)

                hist = psump.tile([P, cap], f32, tag="hist")
                nc.vector.memset(hist[:], 0.0)

                def body(iv):
                    memb = ohp.tile([P, cap], bf16, tag="memb")
                    loh = ohp.tile([P, 64], bf16, tag="loh")
                    nc.vector.tensor_scalar(
                        out=memb[:], in0=candbc[:],
                        scalar1=tc14[:, bass.ds(iv, 1)], scalar2=None,
                        op0=op.is_equal,
                    )
                    nc.vector.tensor_scalar(
                        out=loh[:], in0=iota64_bf[:],
                        scalar1=tlow6[:, bass.ds(iv, 1)], scalar2=None,
                        op0=op.is_equal,
                    )
                    # transposed accumulation: hist[w, cand] += loh^T @ memb,
                    # 512-wide moving slices so the 64-wide stationary loh is
                    # shared and PE streams at full width
                    for g in range(cap // 512):
                        nc.tensor.matmul(
                            out=hist[:64, g * 512 : (g + 1) * 512],
                            lhsT=loh[:],
                            rhs=memb[:, g * 512 : (g + 1) * 512],
                            start=False, stop=True, skip_group_check=True,
                        )

                tc.For_i_unrolled(0, Fl, 1, body, max_unroll=unroll)

                # K = sum over candidates/low6 of [count >= 53]
                kps = kpsp.tile([1, cap], f32, tag="kps")
                for g in range(cap // 512):
                    mask = mkp.tile([P, 512], bf16, tag="mask")
                    nc.vector.tensor_scalar(
                        out=mask[:64, :], in0=hist[:64, g * 512 : (g + 1) * 512],
                        scalar1=52.5, scalar2=None, op0=op.is_ge,
                    )
                    nc.tensor.matmul(
                        out=kps[:1, g * 512 : (g + 1) * 512],
                        lhsT=ones_bf[:64, :], rhs=mask[:64, :],
                        start=True, stop=True,
                    )
                nc.vector.tensor_reduce(
                    out=kv_sb[:1, c : c + 1], in_=kps[:],
                    axis=mybir.AxisListType.X, op=op.add,
                )

            nc.gpsimd.dma_start(out=kvals[:, :], in_=kv_sb[:])
    nc.compile()
    return nc


class _Runner:
    """jit(shard_map(bass_exec)) callable over 8 cores with device-resident
    inputs.  Mirrors concourse.bass2jax.run_bass_via_pjrt's lowering (the
    @via_axon target of bass_utils.run_bass_kernel_spmd), but accepts jax
    Arrays already placed on the devices so repeated launches don't re-ship
    inputs, and keeps the (never-donated, fully-overwritten) output
    parameter slots device-resident too."""

    def __init__(self, nc, n_cores=NCORES):
        import jax
        from concourse import bass2jax
        import concourse.mybir as mybir
        from jax.experimental.shard_map import shard_map
        from jax.sharding import Mesh, PartitionSpec, NamedSharding

        bass2jax.install_neuronx_cc_hook()
        assert not nc.dbg_callbacks if nc.dbg_addr is not None else True
        partition_name = (
            nc.partition_id_tensor.name if nc.partition_id_tensor else None
        )
        self.jax = jax
        self.n_cores = n_cores
        devices = jax.devices()[:n_cores]
        assert len(devices) == n_cores
        self.devices = devices
        self.mesh = Mesh(np.asarray(devices), ("core",))
        self.sharding = NamedSharding(self.mesh, PartitionSpec("core"))

        in_names, out_names, out_avals = [], [], []
        in_meta = {}
        for alloc in nc.m.functions[0].allocations:
            if not isinstance(alloc, mybir.MemoryLocationSet):
                continue
            name = alloc.memorylocations[0].name
            if alloc.kind == "ExternalInput":
                if name == partition_name:
                    continue
                in_names.append(name)
                in_meta[name] = (tuple(alloc.tensor_shape), mybir.dt.np(alloc.dtype))
            elif alloc.kind == "ExternalOutput":
                out_names.append(name)
                out_avals.append(
                    jax.core.ShapedArray(
                        tuple(alloc.tensor_shape), mybir.dt.np(alloc.dtype)
                    )
                )
        self.in_names, self.out_names = in_names, out_names
        self.in_meta = in_meta
        all_in = tuple(in_names) + tuple(out_names)
        if partition_name is not None:
            all_in = all_in + (partition_name,)

        def _body(*args):
            operands = list(args)
            if partition_name is not None:
                operands.append(bass2jax.partition_id_tensor())
            outs = bass2jax._bass_exec_p.bind(
                *operands,
                out_avals=tuple(out_avals),
                in_names=all_in,
                out_names=tuple(out_names),
                lowering_input_output_aliases=(),
                sim_require_finite=True,
                sim_require_nnan=True,
                nc=nc,
            )
            return tuple(outs)

        pspec = PartitionSpec("core")
        n_args = len(in_names) + len(out_names)
        self.fn = jax.jit(
            shard_map(
                _body,
                mesh=self.mesh,
                in_specs=(pspec,) * n_args,
                out_specs=(pspec,) * len(out_names),
                check_rep=False,
            ),
            keep_unused=True,
        )
        # persistent device-resident buffers for the output parameter slots
        # (never donated; the kernels fully overwrite every output element)
        self.out_dummies = [
            jax.device_put(
                np.zeros((n_cores * av.shape[0], *av.shape[1:]), av.dtype),
                self.sharding,
            )
            for av in out_avals
        ]
        self.extra = {}

    def __call__(self, arrays):
        args = []
        for name in self.in_names:
            if name in arrays:
                args.append(arrays[name])
            else:
                if name not in self.extra:
                    shape, dt = self.in_meta[name]
                    z = np.zeros((self.n_cores * shape[0], *shape[1:]), dt)
                    self.extra[name] = self.jax.device_put(z, self.sharding)
                args.append(self.extra[name])
        outs = self.fn(*args, *self.out_dummies)
        return dict(zip(self.out_names, outs))


def _state():
    if "st" in _cache:
        return _cache["st"]
    import jax

    nc1 = build_phase1()
    nc2 = build_phase2()
    run1 = _Runner(nc1)
    run2 = _Runner(nc2)
    st = {
        "jax": jax,
        "run1": run1,
        "run2": run2,
        "devices": run1.devices,
        "sharding": run1.sharding,
        # persistent host work buffers (avoid first-touch page faults on the
        # timed warm call)
        "t": np.empty((B, N, 2), np.float32),
        "sf": np.empty((CLOUDS_PER_CORE, N), np.float32),
        "si": np.empty((CLOUDS_PER_CORE, N), np.int32),
        "ti": np.empty((CLOUDS_PER_CORE, N), np.int32),
        "c14": np.empty((B, N), np.uint16),
        "lo6": np.empty((B, N), np.uint8),
        "w1024": np.asarray([1024.0, 1.0], np.float32),
    }
    _cache["st"] = st
    return st


def _quant_shard(st, pcd_shard, i):
    """Quantize one core-shard [CLOUDS_PER_CORE, N, 2] into the c14/lo6
    plane slices.  Exact: q = rint(1000*p) in f32 (matches jnp.round),
    s = qx*1024 + qz < 2^20 exact in f32."""
    sl = slice(i * CLOUDS_PER_CORE, (i + 1) * CLOUDS_PER_CORE)
    t = st["t"][sl]
    np.multiply(pcd_shard, np.float32(1000.0), out=t)
    np.rint(t, out=t)
    mn = t.min(axis=1)
    mx = t.max(axis=1)
    good = (mn[:, 0] >= 0) & (mn[:, 1] >= 0) & (mx[:, 0] <= 1023) & (mx[:, 1] <= 1023)
    if not good.all():
        t[~good] = 0.0  # keep device indices in range; host recomputes these
    sf, si, ti = st["sf"], st["si"], st["ti"]
    np.dot(t.reshape(-1, 2), st["w1024"], out=sf.reshape(-1))  # s = 1024*qx+qz
    np.copyto(si, sf, casting="unsafe")
    c14, lo6 = st["c14"][sl], st["lo6"][sl]
    np.right_shift(si, 6, out=ti)
    np.copyto(c14, ti, casting="unsafe")
    np.bitwise_and(si, 63, out=ti)
    np.copyto(lo6, ti, casting="unsafe")
    return good


def _host_exact(points):
    """Exact numpy replica of the reference for one cloud. [N,2] f32 -> [TOPK]."""
    q = np.round(np.float32(1000.0) * points.astype(np.float32))
    xi = (q[:, 0] - q[:, 0].min()).astype(np.int64)
    zi = (q[:, 1] - q[:, 1].min()).astype(np.int64)
    idx = xi * GZ + zi
    counts = np.bincount(idx, minlength=1024 * GZ).astype(np.float32)
    occ = counts / np.float32(points.shape[0]) > np.float32(0.0002)
    k = min(int(occ.sum()), TOPK)
    out = np.zeros((TOPK,), np.float32)
    out[:k] = 1.0
    return out


def kernel(pcd):
    import time
    from concurrent.futures import ThreadPoolExecutor

    t_start = time.time()
    pcd = np.ascontiguousarray(np.asarray(pcd), dtype=np.float32)
    assert pcd.shape == (B, N, 2), pcd.shape
    st = _state()
    jax = st["jax"]
    devices = st["devices"]
    _dbg("state ready", t_start)

    shards = pcd.reshape(NCORES, CLOUDS_PER_CORE, N, 2)
    goods = [None] * NCORES
    per_dev = [None] * NCORES  # (c14_dev, lo6_dev) per device

    # pipeline: quantize shard i on the (single) CPU while shard i-1's
    # planes stream over the tunnel from the uploader thread
    def _upload(i):
        sl = slice(i * CLOUDS_PER_CORE, (i + 1) * CLOUDS_PER_CORE)
        c = jax.device_put(st["c14"][sl], devices[i])
        l = jax.device_put(st["lo6"][sl], devices[i])
        return c, l

    with ThreadPoolExecutor(max_workers=2) as ex:
        futs = [None] * NCORES
        for i in range(NCORES):
            goods[i] = _quant_shard(st, shards[i], i)
            futs[i] = ex.submit(_upload, i)
        for i in range(NCORES):
            per_dev[i] = futs[i].result()
    good = np.concatenate(goods)
    _dbg("quantize+upload issued", t_start)

    gshape = (B, N)
    c14_d = jax.make_array_from_single_device_arrays(
        gshape, st["sharding"], [per_dev[i][0] for i in range(NCORES)]
    )
    lo6_d = jax.make_array_from_single_device_arrays(
        gshape, st["sharding"], [per_dev[i][1] for i in range(NCORES)]
    )

    r1 = st["run1"]({"c14p": c14_d})
    h14 = np.asarray(r1["h14"]).reshape(B, P * P)
    _dbg("phase1 done", t_start)

    candmask = h14 >= THRESH_COUNT
    ncand = candmask.sum(1)
    ok = good & (ncand <= CAND_CAP)
    cands = np.full((B, CAND_CAP), -1, np.int16)
    for b in range(B):
        if ok[b]:
            idx = np.flatnonzero(candmask[b])
            cands[b, : len(idx)] = idx.astype(np.int16)
    cands_d = jax.device_put(cands, st["sharding"])
    _dbg("candidates ready", t_start)

    r2 = st["run2"]({"c14p": c14_d, "lo6p": lo6_d, "cands": cands_d})
    kv = np.asarray(r2["kvals"]).reshape(B)
    _dbg("phase2 done", t_start)

    out = np.zeros((B, TOPK, 1), np.float32)
    iota = np.arange(TOPK)
    for b in range(B):
        if ok[b]:
            out[b, :, 0] = iota < kv[b]
        else:
            out[b, :, 0] = _host_exact(pcd[b])
    _dbg("assembled", t_start)
    return out
